# revision 12
# baseline (speedup 1.0000x reference)
"""Fused dual-stream sliding-window attention for Trainium2 (Bass/Tile).

The reference computes two banded softmax streams (s: 0<=i-j<W, c: W<=i-j<2W)
and merges them via LSE. Over disjoint key sets that merge is exactly one
softmax over the union band 0 <= i-j < 2W (W=256), so we compute a single
fused banded attention.

Layout strategy (per (batch, head) pair, sharded 4 pairs/core x 8 cores):
  - host pre-transposes Q, K to [D=128, S] (and casts to bf16) so the kernel
    never transposes
  - per query block b (256 rows), context = key blocks [b-2, b-1, b]
    = 6 chunks of 128 keys, computed in S^T orientation [ck, q]:
        S^T_chunk = matmul(lhsT=K^T[:, chunk], rhs=Q^T[:, block])   # [128, 256]
        p^T = exp(S^T * D^-0.5)        (ACT, scale fused, batched)
        p^T *= triangle mask           (DVE bf16 2x mode, batched)
        out^T accum: matmul(lhsT=p^T[:, half], rhs=V_aug[chunk])    # [128, 130]
    V_aug has ones columns at 128/129 (prefilled host-side) so psum col 128
    accumulates the softmax denominator.
  - normalize with DVE reciprocal + one broadcasted tensor_tensor, DMA out
    (fp32, via GPSIMD's SWDGE ring so stores never block input prefetch).

Matmuls run in bf16 (inputs quantized host-side) with fp32 PSUM accumulation.
The 4 maskable chunks live in one PSUM tile A with slot order [c5 c1 c4 c0],
placing the two all-masked half-tiles at the flat ends, so exp + mask are
single strided ops over the interior; chunks 2/3 (never masked) live in tile
B.  st tiles pack two 1KB chunk outputs per PSUM bank so A+B double-buffered
plus the PV accumulator fit exactly in the 8 banks.  A short burst of dummy
bf16 matmuls at kernel start keeps the PE busy through the initial DMA so the
HAM clock-gate is warm when real work begins.
"""

import ml_dtypes
import numpy as np

import concourse.bass as bass
from concourse import bacc
import concourse.mybir as mybir
import concourse.tile as tile
from concourse.bass_utils import run_bass_kernel_spmd

B, S, H, D = 2, 2048, 16, 128
WIN = 256
N_CORES = 8
PAIRS = (B * H) // N_CORES          # 4 (batch, head) pairs per core
NB = S // WIN                       # 8 query blocks per sequence
SCALE = float(D) ** -0.5
F32 = mybir.dt.float32
BF16 = mybir.dt.bfloat16
NP_BF16 = ml_dtypes.bfloat16
EXP = mybir.ActivationFunctionType.Exp

# chunk -> slot in the single st PSUM tile.  Order [c5 c1 c4 c2 c3 c0] puts
# the two fully-masked half-subtiles (c5 h0, c0 h1) at the flat ends so one
# exp covers the interior, and keeps the maskable stretch c5h1|c1|c4 plus the
# trailing c0h0 aligned with the mask tensor layout [m5 m1 m4 m0].
SLOT = {5: 0, 1: 1, 4: 2, 2: 3, 3: 4, 0: 5}
# (chunk, half) subtiles that are entirely masked out -> skip their PV matmul
EMPTY_SUBTILES = {(0, 1), (5, 0)}
VW = 136          # v tile slot stride (128 data + 2 ones + pad)
N_WARMUP = 220    # dummy matmuls covering the initial DMA to keep HAM warm


def build_masks() -> np.ndarray:
    """0/1 triangle masks in the S^T layout: partition p = key-in-chunk,
    free f = query-in-block.  Valid band: f - p in [128*c - 512, 128*c - 1].
    Slot order matches A_SLOT: chunks 5, 1, 4, 0."""
    p = np.arange(128)[:, None]
    f = np.arange(256)[None, :]
    m = np.zeros((128, 4, 256), np.float32)
    m[:, 0, :] = f >= p + 128     # chunk 5
    m[:, 1, :] = f < p + 128      # chunk 1
    m[:, 2, :] = f >= p           # chunk 4
    m[:, 3, :] = f < p            # chunk 0
    return m.astype(NP_BF16)


def chunks_for_block(b: int) -> list[int]:
    # chunk c of query block b reads key subtile g = 2b - 4 + c; g must be >= 0
    return list(range(max(0, 4 - 2 * b), 6))


def build_program() -> bacc.Bacc:
    nc = bacc.Bacc("TRN2", target_bir_lowering=False, debug=False)

    qt = nc.dram_tensor("qt", [PAIRS, 128, S], BF16, kind="ExternalInput").ap()
    kt = nc.dram_tensor("kt", [PAIRS, 128, S], BF16, kind="ExternalInput").ap()
    vv = nc.dram_tensor("v", [PAIRS, S, 130], BF16, kind="ExternalInput").ap()
    mk = nc.dram_tensor("masks", [128, 4, 256], BF16, kind="ExternalInput").ap()
    out = nc.dram_tensor("out", [PAIRS, S, 128], F32, kind="ExternalOutput").ap()

    with tile.TileContext(nc) as tc:
        with (
            tc.tile_pool(name="const", bufs=1) as const_pool,
            tc.tile_pool(name="qtp", bufs=2 * NB) as qt_pool,
            tc.tile_pool(name="ktp", bufs=8) as kt_pool,
            tc.tile_pool(name="vp", bufs=8) as v_pool,
            tc.tile_pool(name="ptp", bufs=3) as pt_pool,
            tc.tile_pool(name="stp", bufs=2, space="PSUM") as st_pool,
            tc.tile_pool(name="pv", bufs=2, space="PSUM") as pv_pool,
            tc.tile_pool(name="outp", bufs=4) as out_pool,
            tc.tile_pool(name="rcp", bufs=4) as rcp_pool,
        ):
            mask_sb = const_pool.tile([128, 4, 256], BF16)
            nc.sync.dma_start(mask_sb[:], mk[:])

            # PE warm-up: harmless matmuls on the mask tile while the first
            # pair's DMAs land, so HAM reaches K=8/8 before real work; the
            # psum results are never read (next start=True resets).
            wpsum = pv_pool.tile([128, 2, VW], F32, tag="pv")
            for _ in range(N_WARMUP):
                nc.tensor.matmul(wpsum[:, 0, 0:32], lhsT=mask_sb[:, 0, 0:128],
                                 rhs=mask_sb[:, 0, 0:32], start=True, stop=True)

            for pair in range(PAIRS):
                # inputs split into pieces ordered by first use so compute
                # starts as soon as the early pieces land
                qt_t, kt_t, v_t = [], [], []

                def load_piece(j, pair=pair, kt_t=kt_t, v_t=v_t):
                    k_tile = kt_pool.tile([128, 512], BF16)
                    nc.sync.dma_start(k_tile[:],
                                      kt[pair, :, j * 512:(j + 1) * 512])
                    kt_t.append(k_tile)
                    vt = v_pool.tile([128, 4, VW], BF16)
                    nc.sync.dma_start(
                        vt[:, :, 0:130],
                        vv[pair, j * 512:(j + 1) * 512, :].rearrange(
                            "(g p) d -> p g d", p=128),
                    )
                    v_t.append(vt)

                def load_q(j, pair=pair, qt_t=qt_t):
                    q_tile = qt_pool.tile([128, 512], BF16)
                    nc.sync.dma_start(q_tile[:],
                                      qt[pair, :, j * 512:(j + 1) * 512])
                    qt_t.append(q_tile)

                load_q(0)
                load_piece(0)
                load_q(1)
                load_piece(1)
                load_q(2)
                load_piece(2)
                load_q(3)
                load_piece(3)

                for b in range(NB):
                    cs = chunks_for_block(b)

                    st = st_pool.tile([128, 6, 256], F32)
                    for c in cs:
                        g = 2 * b - 4 + c
                        nc.tensor.matmul(
                            st[:, SLOT[c], :],
                            lhsT=kt_t[g // 4][:, (g % 4) * 128:(g % 4 + 1) * 128],
                            rhs=qt_t[b // 2][:, (b % 2) * 256:(b % 2 + 1) * 256],
                            start=True, stop=True,
                        )

                    pT = pt_pool.tile([128, 6, 256], BF16)
                    st_f = st[:].rearrange("p a f -> p (a f)")
                    pT_f = pT[:].rearrange("p a f -> p (a f)")
                    mk_f = mask_sb[:].rearrange("p a f -> p (a f)")
                    if b >= 2:
                        # all chunks present: one exp over the contiguous
                        # interior [c5h1 c1 c4 c2 c3 c0h0]; the flat ends are
                        # the fully-masked halves and are never read
                        nc.scalar.activation(pT_f[:, 128:1408],
                                             st_f[:, 128:1408], EXP, scale=SCALE)
                        nc.vector.tensor_mul(pT_f[:, 128:768],
                                             pT_f[:, 128:768], mk_f[:, 128:768])
                        nc.vector.tensor_mul(pT_f[:, 1280:1408],
                                             pT_f[:, 1280:1408],
                                             mk_f[:, 768:896])
                    else:
                        # b=0: chunks 4,5; b=1: chunks 2..5
                        hi = 768 if b == 0 else 1280
                        nc.scalar.activation(pT_f[:, 128:256],
                                             st_f[:, 128:256], EXP, scale=SCALE)
                        nc.vector.tensor_mul(pT_f[:, 128:256],
                                             pT_f[:, 128:256],
                                             mk_f[:, 128:256])
                        nc.scalar.activation(pT_f[:, 512:hi],
                                             st_f[:, 512:hi], EXP, scale=SCALE)
                        nc.vector.tensor_mul(pT_f[:, 512:768],
                                             pT_f[:, 512:768], mk_f[:, 512:768])

                    pv = pv_pool.tile([128, 2, VW], F32, tag="pv")
                    for h in (0, 1):
                        mms = [c for c in (2, 3, 0, 1, 4, 5)
                               if c in cs and (c, h) not in EMPTY_SUBTILES]
                        for i, c in enumerate(mms):
                            g = 2 * b - 4 + c
                            nc.tensor.matmul(
                                pv[:, h, 0:130],
                                lhsT=pT[:, SLOT[c], h * 128:(h + 1) * 128],
                                rhs=v_t[g // 4][:, g % 4, 0:130],
                                start=(i == 0), stop=(i == len(mms) - 1),
                            )

                    recip = rcp_pool.tile([128, 2], F32)
                    nc.vector.reciprocal(recip[:], pv[:, :, 128])
                    ot = out_pool.tile([128, 2, 128], F32)
                    nc.vector.tensor_mul(
                        ot[:], pv[:, :, 0:128],
                        recip[:].unsqueeze(2).broadcast_to([128, 2, 128]),
                    )
                    nc.gpsimd.dma_start(
                        out[pair, b * 256:(b + 1) * 256, :].rearrange(
                            "(h p) d -> p h d", h=2),
                        ot[:],
                    )
    nc.compile()
    return nc


_CACHE: dict = {}


def _get_program() -> bacc.Bacc:
    if "nc" not in _CACHE:
        _CACHE["nc"] = build_program()
    return _CACHE["nc"]


def make_in_maps(query, key, value):
    """Shard + pre-transpose full [B,S,H,D] inputs into per-core input maps."""
    qt_all = query.transpose(0, 2, 3, 1).astype(NP_BF16)   # [B,H,D,S]
    kt_all = key.transpose(0, 2, 3, 1).astype(NP_BF16)
    v_all = np.empty((B, H, S, 130), NP_BF16)              # [B,H,S,D+2ones]
    v_all[:, :, :, 0:128] = value.transpose(0, 2, 1, 3).astype(NP_BF16)
    v_all[:, :, :, 128:130] = 1.0
    masks = build_masks()
    in_maps = []
    for c in range(N_CORES):
        idx = [divmod(c * PAIRS + i, H) for i in range(PAIRS)]
        in_maps.append({
            "qt": np.ascontiguousarray(np.stack([qt_all[b, h] for b, h in idx])),
            "kt": np.ascontiguousarray(np.stack([kt_all[b, h] for b, h in idx])),
            "v": np.ascontiguousarray(np.stack([v_all[b, h] for b, h in idx])),
            "masks": masks,
        })
    return in_maps


def gather_output(results) -> np.ndarray:
    out = np.empty((B, S, H, D), np.float32)
    for c in range(N_CORES):
        o = results[c]["out"]
        for i in range(PAIRS):
            b, h = divmod(c * PAIRS + i, H)
            out[b, :, h, :] = o[i]
    return out


def run(query, key, value, trace: bool = False):
    nc = _get_program()
    in_maps = make_in_maps(query, key, value)
    res = run_bass_kernel_spmd(nc, in_maps, core_ids=list(range(N_CORES)),
                               trace=trace)
    return gather_output(res.results), res


def kernel(query, key, value):
    out, _ = run(query, key, value)
    return out


# revision 13
# speedup vs baseline: 1.0041x; 1.0041x over previous
"""Fused dual-stream sliding-window attention for Trainium2 (Bass/Tile).

The reference computes two banded softmax streams (s: 0<=i-j<W, c: W<=i-j<2W)
and merges them via LSE. Over disjoint key sets that merge is exactly one
softmax over the union band 0 <= i-j < 2W (W=256), so we compute a single
fused banded attention.

Layout strategy (per (batch, head) pair, sharded 4 pairs/core x 8 cores):
  - host pre-transposes Q, K to [D=128, S] (and casts to bf16) so the kernel
    never transposes
  - per query block b (256 rows), context = key blocks [b-2, b-1, b]
    = 6 chunks of 128 keys, computed in S^T orientation [ck, q]:
        S^T_chunk = matmul(lhsT=K^T[:, chunk], rhs=Q^T[:, block])   # [128, 256]
        p^T = exp(S^T * D^-0.5)        (ACT, scale fused, batched)
        p^T *= triangle mask           (DVE bf16 2x mode, batched)
        out^T accum: matmul(lhsT=p^T[:, half], rhs=V_aug[chunk])    # [128, 130]
    V_aug has ones columns at 128/129 (prefilled host-side) so psum col 128
    accumulates the softmax denominator.
  - normalize with DVE reciprocal + one broadcasted tensor_tensor, DMA out
    (fp32, via GPSIMD's SWDGE ring so stores never block input prefetch).

Matmuls run in bf16 (inputs quantized host-side) with fp32 PSUM accumulation.
The 4 maskable chunks live in one PSUM tile A with slot order [c5 c1 c4 c0],
placing the two all-masked half-tiles at the flat ends, so exp + mask are
single strided ops over the interior; chunks 2/3 (never masked) live in tile
B.  st tiles pack two 1KB chunk outputs per PSUM bank so A+B double-buffered
plus the PV accumulator fit exactly in the 8 banks.  A short burst of dummy
bf16 matmuls at kernel start keeps the PE busy through the initial DMA so the
HAM clock-gate is warm when real work begins.
"""

import ml_dtypes
import numpy as np

import concourse.bass as bass
from concourse import bacc
import concourse.mybir as mybir
import concourse.tile as tile
from concourse.bass_utils import run_bass_kernel_spmd

B, S, H, D = 2, 2048, 16, 128
WIN = 256
N_CORES = 8
PAIRS = (B * H) // N_CORES          # 4 (batch, head) pairs per core
NB = S // WIN                       # 8 query blocks per sequence
SCALE = float(D) ** -0.5
F32 = mybir.dt.float32
BF16 = mybir.dt.bfloat16
NP_BF16 = ml_dtypes.bfloat16
EXP = mybir.ActivationFunctionType.Exp

# chunk -> slot in the single st PSUM tile.  Order [c5 c1 c4 c2 c3 c0] puts
# the two fully-masked half-subtiles (c5 h0, c0 h1) at the flat ends so one
# exp covers the interior, and keeps the maskable stretch c5h1|c1|c4 plus the
# trailing c0h0 aligned with the mask tensor layout [m5 m1 m4 m0].
SLOT = {5: 0, 1: 1, 4: 2, 2: 3, 3: 4, 0: 5}
# (chunk, half) subtiles that are entirely masked out -> skip their PV matmul
EMPTY_SUBTILES = {(0, 1), (5, 0)}
VW = 136          # v tile slot stride (128 data + 2 ones + pad)
N_WARMUP = 220    # dummy matmuls covering the initial DMA to keep HAM warm


def build_masks() -> np.ndarray:
    """0/1 triangle masks in the S^T layout: partition p = key-in-chunk,
    free f = query-in-block.  Valid band: f - p in [128*c - 512, 128*c - 1].
    Slot order matches A_SLOT: chunks 5, 1, 4, 0."""
    p = np.arange(128)[:, None]
    f = np.arange(256)[None, :]
    m = np.zeros((128, 4, 256), np.float32)
    m[:, 0, :] = f >= p + 128     # chunk 5
    m[:, 1, :] = f < p + 128      # chunk 1
    m[:, 2, :] = f >= p           # chunk 4
    m[:, 3, :] = f < p            # chunk 0
    return m.astype(NP_BF16)


def chunks_for_block(b: int) -> list[int]:
    # chunk c of query block b reads key subtile g = 2b - 4 + c; g must be >= 0
    return list(range(max(0, 4 - 2 * b), 6))


def build_program() -> bacc.Bacc:
    nc = bacc.Bacc("TRN2", target_bir_lowering=False, debug=False)

    qt = nc.dram_tensor("qt", [PAIRS, 128, S], BF16, kind="ExternalInput").ap()
    kt = nc.dram_tensor("kt", [PAIRS, 128, S], BF16, kind="ExternalInput").ap()
    vv = nc.dram_tensor("v", [PAIRS, S, 130], BF16, kind="ExternalInput").ap()
    mk = nc.dram_tensor("masks", [128, 4, 256], BF16, kind="ExternalInput").ap()
    out = nc.dram_tensor("out", [PAIRS, S, 128], F32, kind="ExternalOutput").ap()

    with tile.TileContext(nc) as tc:
        with (
            tc.tile_pool(name="const", bufs=1) as const_pool,
            tc.tile_pool(name="qtp", bufs=2 * NB) as qt_pool,
            tc.tile_pool(name="ktp", bufs=8) as kt_pool,
            tc.tile_pool(name="vp", bufs=8) as v_pool,
            tc.tile_pool(name="ptp", bufs=3) as pt_pool,
            tc.tile_pool(name="stp", bufs=2, space="PSUM") as st_pool,
            tc.tile_pool(name="pv", bufs=2, space="PSUM") as pv_pool,
            tc.tile_pool(name="outp", bufs=4) as out_pool,
            tc.tile_pool(name="rcp", bufs=4) as rcp_pool,
        ):
            mask_sb = const_pool.tile([128, 4, 256], BF16)
            nc.sync.dma_start(mask_sb[:], mk[:])

            # PE warm-up: harmless matmuls on the mask tile while the first
            # pair's DMAs land, so HAM reaches K=8/8 before real work; the
            # psum results are never read (next start=True resets).
            wpsum = pv_pool.tile([128, 2, VW], F32, tag="pv")
            for _ in range(N_WARMUP):
                nc.tensor.matmul(wpsum[:, 0, 0:32], lhsT=mask_sb[:, 0, 0:128],
                                 rhs=mask_sb[:, 0, 0:32], start=True, stop=True)

            def emit_st_exp_mask(pair, b, qt_t, kt_t):
                """S^T matmuls + batched exp + mask for one query block."""
                cs = chunks_for_block(b)
                st = st_pool.tile([128, 6, 256], F32, tag="st")
                for c in cs:
                    g = 2 * b - 4 + c
                    nc.tensor.matmul(
                        st[:, SLOT[c], :],
                        lhsT=kt_t[g // 4][:, (g % 4) * 128:(g % 4 + 1) * 128],
                        rhs=qt_t[b // 2][:, (b % 2) * 256:(b % 2 + 1) * 256],
                        start=True, stop=True,
                    )
                pT = pt_pool.tile([128, 6, 256], BF16, tag="pT")
                st_f = st[:].rearrange("p a f -> p (a f)")
                pT_f = pT[:].rearrange("p a f -> p (a f)")
                mk_f = mask_sb[:].rearrange("p a f -> p (a f)")
                if b >= 2:
                    # all chunks present: one exp over the contiguous interior
                    # [c5h1 c1 c4 c2 c3 c0h0]; the flat ends are the
                    # fully-masked halves and are never read
                    nc.scalar.activation(pT_f[:, 128:1408],
                                         st_f[:, 128:1408], EXP, scale=SCALE)
                    nc.vector.tensor_mul(pT_f[:, 128:768],
                                         pT_f[:, 128:768], mk_f[:, 128:768])
                    nc.vector.tensor_mul(pT_f[:, 1280:1408],
                                         pT_f[:, 1280:1408], mk_f[:, 768:896])
                else:
                    # b=0: chunks 4,5; b=1: chunks 2..5
                    hi = 768 if b == 0 else 1280
                    nc.scalar.activation(pT_f[:, 128:256],
                                         st_f[:, 128:256], EXP, scale=SCALE)
                    nc.vector.tensor_mul(pT_f[:, 128:256],
                                         pT_f[:, 128:256], mk_f[:, 128:256])
                    nc.scalar.activation(pT_f[:, 512:hi],
                                         st_f[:, 512:hi], EXP, scale=SCALE)
                    nc.vector.tensor_mul(pT_f[:, 512:768],
                                         pT_f[:, 512:768], mk_f[:, 512:768])
                return pT

            def emit_pv_norm_out(pair, b, pT, v_t):
                """PV accumulation, normalize, store for one query block."""
                cs = chunks_for_block(b)
                pv = pv_pool.tile([128, 2, VW], F32, tag="pv")
                for h in (0, 1):
                    mms = [c for c in (2, 3, 0, 1, 4, 5)
                           if c in cs and (c, h) not in EMPTY_SUBTILES]
                    for i, c in enumerate(mms):
                        g = 2 * b - 4 + c
                        nc.tensor.matmul(
                            pv[:, h, 0:130],
                            lhsT=pT[:, SLOT[c], h * 128:(h + 1) * 128],
                            rhs=v_t[g // 4][:, g % 4, 0:130],
                            start=(i == 0), stop=(i == len(mms) - 1),
                        )
                recip = rcp_pool.tile([128, 2], F32)
                nc.vector.reciprocal(recip[:], pv[:, :, 128])
                ot = out_pool.tile([128, 2, 128], F32)
                nc.vector.tensor_mul(
                    ot[:], pv[:, :, 0:128],
                    recip[:].unsqueeze(2).broadcast_to([128, 2, 128]),
                )
                nc.gpsimd.dma_start(
                    out[pair, b * 256:(b + 1) * 256, :].rearrange(
                        "(h p) d -> p h d", h=2),
                    ot[:],
                )

            # software-pipelined by one query block: the PV matmuls of block
            # b-1 are emitted after the st matmuls of block b, so the PE
            # crunches PV(b-1) while ACT runs exp(b); carried across pairs.
            pending = None
            for pair in range(PAIRS):
                qt_t, kt_t, v_t = [], [], []

                def load_piece(j, pair=pair, kt_t=kt_t, v_t=v_t):
                    k_tile = kt_pool.tile([128, 512], BF16)
                    nc.sync.dma_start(k_tile[:],
                                      kt[pair, :, j * 512:(j + 1) * 512])
                    kt_t.append(k_tile)
                    vt = v_pool.tile([128, 4, VW], BF16)
                    nc.sync.dma_start(
                        vt[:, :, 0:130],
                        vv[pair, j * 512:(j + 1) * 512, :].rearrange(
                            "(g p) d -> p g d", p=128),
                    )
                    v_t.append(vt)

                def load_q(j, pair=pair, qt_t=qt_t):
                    q_tile = qt_pool.tile([128, 512], BF16)
                    nc.sync.dma_start(q_tile[:],
                                      qt[pair, :, j * 512:(j + 1) * 512])
                    qt_t.append(q_tile)

                load_q(0)
                load_piece(0)
                load_q(1)
                load_piece(1)
                load_q(2)
                load_piece(2)
                load_q(3)
                load_piece(3)

                for b in range(NB):
                    pT = emit_st_exp_mask(pair, b, qt_t, kt_t)
                    if pending is not None:
                        emit_pv_norm_out(*pending)
                    pending = (pair, b, pT, v_t)
            emit_pv_norm_out(*pending)

    nc.compile()
    return nc


_CACHE: dict = {}


def _get_program() -> bacc.Bacc:
    if "nc" not in _CACHE:
        _CACHE["nc"] = build_program()
    return _CACHE["nc"]


def make_in_maps(query, key, value):
    """Shard + pre-transpose full [B,S,H,D] inputs into per-core input maps."""
    qt_all = query.transpose(0, 2, 3, 1).astype(NP_BF16)   # [B,H,D,S]
    kt_all = key.transpose(0, 2, 3, 1).astype(NP_BF16)
    v_all = np.empty((B, H, S, 130), NP_BF16)              # [B,H,S,D+2ones]
    v_all[:, :, :, 0:128] = value.transpose(0, 2, 1, 3).astype(NP_BF16)
    v_all[:, :, :, 128:130] = 1.0
    masks = build_masks()
    in_maps = []
    for c in range(N_CORES):
        idx = [divmod(c * PAIRS + i, H) for i in range(PAIRS)]
        in_maps.append({
            "qt": np.ascontiguousarray(np.stack([qt_all[b, h] for b, h in idx])),
            "kt": np.ascontiguousarray(np.stack([kt_all[b, h] for b, h in idx])),
            "v": np.ascontiguousarray(np.stack([v_all[b, h] for b, h in idx])),
            "masks": masks,
        })
    return in_maps


def gather_output(results) -> np.ndarray:
    out = np.empty((B, S, H, D), np.float32)
    for c in range(N_CORES):
        o = results[c]["out"]
        for i in range(PAIRS):
            b, h = divmod(c * PAIRS + i, H)
            out[b, :, h, :] = o[i]
    return out


def run(query, key, value, trace: bool = False):
    nc = _get_program()
    in_maps = make_in_maps(query, key, value)
    res = run_bass_kernel_spmd(nc, in_maps, core_ids=list(range(N_CORES)),
                               trace=trace)
    return gather_output(res.results), res


def kernel(query, key, value):
    out, _ = run(query, key, value)
    return out
